# revision 31
# baseline (speedup 1.0000x reference)
"""GraphSAGE 2-layer forward on 8 Trainium2 NeuronCores (Bass kernel).

Strategy
--------
Math reordering: mean-aggregation commutes with the linear layers, so we
project first (y1 = x @ W1_l.T, 8 cols) and aggregate 8-wide messages
instead of 128-wide ones.

Sharding (dst-parallel): node n -> core n % 8 (keeps edge counts balanced
to ~0.2%).  Within a core, nodes are degree-sorted and dealt round-robin
onto the 128 SBUF partitions, so every partition holds the same
per-position degree sequence (padded to the max over all 1024 streams;
~2% overhead).  Each partition's edge stream is then a concatenation of
fixed-width runs, and the per-node segment-sum becomes ~21 regular
strided tensor_reduce instructions -- no scatter anywhere.

Per-edge data movement uses the only HW-correct indirect-DMA form in
this toolchain: 128 indices per instruction (one per partition), one
32-byte row each.  Inter-layer tables (y1, h; 400KB/core slices) are
shared via AllGather.  The tiny linear layers run on the PE; x is staged
host-pretransposed in bf16 so the projection needs no on-device
transposes, and the final linear runs on a 4-deep PSUM rotation.

Per-core pipeline: x load -> PE project -> AllGather y1 -> S indirect
gathers + interleaved run-reduces -> h = relu(...) -> AllGather h ->
S gathers + run-reduces -> PE final linear -> out.
"""

import numpy as np
import ml_dtypes

N = 100000
E_DIM = 128
F = 8
NCLS = 40
C = 8                 # cores
PP = 98               # node positions per partition
PC = 128 * PP         # node slots per core (12544)
NPAD = C * PC         # 100352
GATHER_LAG = 32       # sem margin for out-of-order DMA completions

_RUN_CACHE = {}


# ----------------------------------------------------------------------------
# host preprocessing: node permutation, edge slots, run structure
# ----------------------------------------------------------------------------
def _preprocess(src, dst):
    cnt = np.bincount(dst, minlength=N).astype(np.int64)
    cntp = np.zeros(NPAD, np.int64)
    cntp[:N] = cnt

    # per core: degree-sorted rank r; device slot m = (r%128)*PP + r//128
    orig_of = np.empty((C, PC), np.int64)     # [core, rank] -> original id
    g_of = np.empty(NPAD, np.int64)           # original id -> global m-index
    r = np.arange(PC)
    m_of_rank = (r % 128) * PP + r // 128
    for c in range(C):
        ids = np.arange(c, NPAD, C)
        order = np.argsort(cntp[ids], kind="stable")
        ids_sorted = ids[order]
        orig_of[c] = ids_sorted
        g_of[ids_sorted] = c * PC + m_of_rank

    # per-position padded width, unified over all cores/partitions
    deg_sorted = cntp[orig_of.reshape(-1)].reshape(C, PP, 128)  # [c, i, p]
    w = deg_sorted.max(axis=(0, 2)).astype(np.int64)            # [PP]
    S = int(w.sum())
    t0 = np.zeros(PP + 1, np.int64)
    np.cumsum(w, out=t0[1:])

    runs = []
    i = 0
    while i < PP:
        j = i
        while j < PP and w[j] == w[i]:
            j += 1
        if w[i] > 0:
            runs.append((int(i), int(j), int(w[i])))
        i = j

    # per-edge slot assignment
    g_dst = g_of[dst]
    order = np.argsort(g_dst)
    sg = g_dst[order]
    gsrc = g_of[src[order]].astype(np.int32)
    newgrp = np.empty(sg.shape[0], bool)
    newgrp[0] = True
    np.not_equal(sg[1:], sg[:-1], out=newgrp[1:])
    first_pos = np.flatnonzero(newgrp)
    grp = np.cumsum(newgrp) - 1
    k = np.arange(sg.shape[0]) - first_pos[grp]

    c_e = sg // PC
    m_e = sg % PC
    p_e = m_e // PP
    i_e = m_e % PP
    t_e = t0[i_e] + k

    # zero row: core 0's first pad node, in m-encoding
    pad_rank0 = int(np.flatnonzero(orig_of[0] >= N)[0])
    zero_g = int(m_of_rank[pad_rank0])        # core 0 base is 0

    idx = np.full((C, 128, S), zero_g, np.int32)
    idx[c_e, p_e, t_e] = gsrc

    inv = (1.0 / np.maximum(cntp, 1.0)).astype(np.float32)
    inv_cm = inv[orig_of]                                   # [C, rank]
    inv8 = np.ascontiguousarray(
        inv_cm.reshape(C, PP, 128).transpose(0, 2, 1))[..., None]  # [C,p,i,1]
    inv8 = np.ascontiguousarray(np.broadcast_to(inv8, (C, 128, PP, F)),
                                dtype=np.float32)

    return dict(S=S, runs=runs, t0=t0, idx=idx, inv8=inv8,
                orig_of=orig_of, zero_g=zero_g)


# ----------------------------------------------------------------------------
# program builder (raw Bass, explicit semaphore plan)
# ----------------------------------------------------------------------------
class _Op:
    __slots__ = ("emit", "waits", "incs")

    def __init__(self, emit, waits, incs):
        self.emit, self.waits, self.incs = emit, waits, incs


class _Plan:
    def __init__(self):
        self.ops = {k: [] for k in ("gp", "sync", "pe", "ve")}
        self.ct = {}

    def n(self, sem):
        return self.ct.get(sem, 0)

    def add(self, eng, emit, waits=(), incs=()):
        for sem, cnt in incs:
            self.ct[sem] = self.ct.get(sem, 0) + cnt
        self.ops[eng].append(_Op(emit, list(waits), list(incs)))
        return {sem: self.ct[sem] for sem, _ in incs}


def _build(S, runs, t0, zero_g, K, with_b1, with_b2, dbg=False):
    from concourse import bass, mybir
    from concourse.bass import IndirectOffsetOnAxis

    nc = bass.Bass(target_bir_lowering=False, debug=False)
    f32, i32 = mybir.dt.float32, mybir.dt.int32
    bf16 = mybir.dt.bfloat16

    x_in = nc.declare_dram_parameter("xt", [128, PP * 128], bf16, isOutput=False)
    idx_in = nc.declare_dram_parameter("idx", [128, S], i32, isOutput=False)
    inv_in = nc.declare_dram_parameter("inv", [128, PP, F], f32, isOutput=False)
    w1_in = nc.declare_dram_parameter("w1cat", [E_DIM, 16], bf16, isOutput=False)
    w2l_in = nc.declare_dram_parameter("w2l", [8, NCLS], f32, isOutput=False)
    w2r_in = nc.declare_dram_parameter("w2r", [8, NCLS], f32, isOutput=False)
    id_in = nc.declare_dram_parameter("ident", [128, 128], f32, isOutput=False)
    z_in = nc.declare_dram_parameter("zrow", [1, F], f32, isOutput=False)
    b1_in = b2_in = None
    if with_b1:
        b1_in = nc.declare_dram_parameter("b1rep", [128, PP, F], f32, isOutput=False)
    if with_b2:
        b2_in = nc.declare_dram_parameter("b2rep", [128, NCLS], f32, isOutput=False)
    out_ext = nc.declare_dram_parameter("out", [PC, NCLS], f32, isOutput=True)
    if dbg:
        dy = nc.declare_dram_parameter("dy", [PC, F], f32, isOutput=True)
        dh = nc.declare_dram_parameter("dh", [PC, F], f32, isOutput=True)
        da1 = nc.declare_dram_parameter("da1", [PC, F], f32, isOutput=True)
        da2 = nc.declare_dram_parameter("da2", [PC, F], f32, isOutput=True)
        dg1 = nc.declare_dram_parameter("dg1", [128 * (S // 8), F], f32, isOutput=True)

    y_slice = nc.dram_tensor("y_slice", [PC, F], f32)
    h_slice = nc.dram_tensor("h_slice", [PC, F], f32)
    y_full = nc.dram_tensor("y_full", [NPAD, F], f32, addr_space="Shared")
    h_full = nc.dram_tensor("h_full", [NPAD, F], f32, addr_space="Shared")

    P = _Plan()
    DG, DS, CC, PE, VE = "dg", "ds", "cc", "pe", "ve"

    from contextlib import ExitStack
    with ExitStack() as ctx:
        ec = ctx.enter_context
        block = ec(nc.Block())
        dg_s = ec(nc.semaphore("dg"))
        ds_s = ec(nc.semaphore("ds"))
        cc_s = ec(nc.semaphore("cc"))
        pe_s = ec(nc.semaphore("pe"))
        ve_s = ec(nc.semaphore("ve"))
        x_sb = ec(nc.sbuf_tensor("x_sb", [128, PP, 128], bf16))
        G = ec(nc.sbuf_tensor("G", [128, S, F], f32))
        idx_sb = ec(nc.sbuf_tensor("idx_sb", [128, S], i32))
        inv_sb = ec(nc.sbuf_tensor("inv_sb", [128, PP, F], f32))
        y1l = ec(nc.sbuf_tensor("y1l", [128, PP, F], f32))
        y1r = ec(nc.sbuf_tensor("y1r", [128, PP, F], f32))
        agg = ec(nc.sbuf_tensor("agg", [128, PP, F], f32))
        hbuf = ec(nc.sbuf_tensor("hbuf", [128, PP, F], f32))
        htmp = ec(nc.sbuf_tensor("htmp", [128, PP, F], f32))
        htmp2 = ec(nc.sbuf_tensor("htmp2", [128, PP, F], f32))
        out_sb = ec(nc.sbuf_tensor("out_sb", [128, PP, NCLS], f32))
        aTcs = [ec(nc.sbuf_tensor(f"aTc{m}", [8, 128], f32)) for m in range(4)]
        hTcs = [ec(nc.sbuf_tensor(f"hTc{m}", [8, 128], f32)) for m in range(4)]
        w1_sb = ec(nc.sbuf_tensor("w1_sb", [128, 16], bf16))
        w2l_sb = ec(nc.sbuf_tensor("w2l_sb", [8, NCLS], f32))
        w2r_sb = ec(nc.sbuf_tensor("w2r_sb", [8, NCLS], f32))
        id_sb = ec(nc.sbuf_tensor("id_sb", [128, 128], f32))
        z_sb = ec(nc.sbuf_tensor("z_sb", [1, F], f32))
        b1_sb = ec(nc.sbuf_tensor("b1_sb",
                                  [128, PP, F] if with_b1 else [1, F], f32))
        b2_sb = ec(nc.sbuf_tensor("b2_sb",
                                  [128, NCLS] if with_b2 else [1, F], f32))
        psts = [ec(nc.psum_tensor(f"pst{m}", [8, 128], f32)) for m in range(4)]
        psos = [ec(nc.psum_tensor(f"pso{m}", [128, NCLS], f32))
                for m in range(4)]
        sems = {DG: dg_s, DS: ds_s, CC: cc_s, PE: pe_s, VE: ve_s}
        aTc = aTcs
        hTc = hTcs
        psy = [psos[0], psos[1]]   # column-aliased: phases are disjoint
        pst = psts
        pso = psos

        def gdma(out, in_):
            return lambda g: g.dma_start(out=out, in_=in_)

        # ---- static loads -------------------------------------------------
        static_tiles = [
            (idx_sb[:, :], idx_in[:, :]),
            (inv_sb[:, :, :], inv_in[:, :, :]),
            (w1_sb[:, :], w1_in[:, :]),
            (w2l_sb[:, :], w2l_in[:, :]),
            (w2r_sb[:, :], w2r_in[:, :]),
            (id_sb[:, :], id_in[:, :]),
            (z_sb[:, :], z_in[:, :]),
        ]
        if with_b1:
            static_tiles.append((b1_sb[:, :, :], b1_in[:, :, :]))
        if with_b2:
            static_tiles.append((b2_sb[:, :], b2_in[:, :]))
        for out_t, in_t in static_tiles:
            P.add("gp", gdma(out_t, in_t), incs=[(DG, 16)])
        P.add("sync", gdma(x_sb[:, :, :],
                           x_in[:, :].rearrange("f (i n) -> f i n", i=PP)),
              incs=[(DS, 16)])
        static_dg = P.n(DG)
        static_ds = P.n(DS)

        def transpose_op(dst_ps, src_ap):
            return lambda t: t.matmul(out=dst_ps, lhsT=src_ap, rhs=id_sb[:, :],
                                      is_transpose=True, start=True, stop=True)

        def coll(inp, outp):
            return lambda g: g.collective_compute(
                "AllGather", mybir.AluOpType.bypass,
                replica_groups=[list(range(C))],
                ins=[inp.ap().opt()], outs=[outp.ap().opt()])

        prev_end = {DG: static_dg, DS: static_ds, PE: 0, VE: 0, CC: 0}

        for _it in range(K):
            # ---- body-start barrier --------------------------------------
            bar = [(s, prev_end[s]) for s in (DG, DS, PE, VE, CC)]
            for eng in ("gp", "sync", "pe", "ve"):
                P.add(eng, None, waits=bar)

            # ---- phase 2: project x (PE/VE pipeline) ---------------------
            pe_base = P.n(PE)
            ve_base = P.n(VE)
            for i in range(PP):
                waits_t = [(DS, static_ds)]
                if i == 0:
                    waits_t.append((DG, static_dg))
                if i >= 2:
                    waits_t.append((VE, ve_base + 2 * (i - 2) + 2))

                def mm_y(i=i):
                    return lambda t: t.matmul(out=psy[i % 2][:, 0:16],
                                              lhsT=x_sb[:, i, :],
                                              rhs=w1_sb[:, :],
                                              start=True, stop=True)
                P.add("pe", mm_y(), waits=waits_t, incs=[(PE, 1)])

                def cp_yl(i=i):
                    return lambda v: v.tensor_copy(out=y1l[:, i, :],
                                                   in_=psy[i % 2][:, 0:8])
                def cp_yr(i=i):
                    return lambda v: v.tensor_copy(out=y1r[:, i, :],
                                                   in_=psy[i % 2][:, 8:16])
                P.add("ve", cp_yl(), waits=[(PE, pe_base + i + 1)],
                      incs=[(VE, 1)])
                P.add("ve", cp_yr(), incs=[(VE, 1)])

            P.add("ve", lambda v: (v.memset(agg[:, :, :], 0.0), v.drain())[0],
                  incs=[(VE, 1)])
            proj_ve = P.n(VE)

            # ---- phase 3: y writeback + AllGather + zero-fix -------------
            wy = P.add("sync",
                       gdma(y_slice[:, :].rearrange("(p i) f -> p i f", p=128),
                            y1l[:, :, :]),
                       waits=[(VE, proj_ve)], incs=[(DS, 16)])
            wc = P.add("gp", coll(y_slice, y_full), waits=[(DS, wy[DS])],
                       incs=[(CC, 1)])
            P.add("gp", gdma(y_full[zero_g:zero_g + 1, :], z_sb[:, :]),
                  waits=[(CC, wc[CC])], incs=[(DG, 16)])

            # ---- gather+reduce helper ------------------------------------
            def gather_layer(table, pre_ve_wait):
                base_dg = P.n(DG)
                for t in range(S):
                    def gop(t=t, table=table):
                        return lambda g: g.indirect_dma_start(
                            out=G[:, t, :], out_offset=None, in_=table[:, :],
                            in_offset=IndirectOffsetOnAxis(
                                ap=idx_sb[:, t:t + 1], axis=0))
                    wts = [(DG, base_dg), (VE, pre_ve_wait)] if t == 0 else []
                    P.add("gp", gop(), waits=wts, incs=[(DG, 16)])
                red = []
                for (i0, i1, wd) in runs:
                    thresh = base_dg + 16 * min(S, int(t0[i1]) + GATHER_LAG)
                    def rop(i0=i0, i1=i1, wd=wd):
                        lo, hi = int(t0[i0]), int(t0[i1])
                        def f(v):
                            src = G[:, lo:hi, :].rearrange(
                                "p (i w) f -> p i f w", w=wd)
                            return v.tensor_reduce(
                                out=agg[:, i0:i1, :], in_=src,
                                axis=mybir.AxisListType.X,
                                op=mybir.AluOpType.add)
                        return f
                    rc = P.add("ve", rop(), waits=[(DG, thresh)], incs=[(VE, 1)])
                    red.append((i0, i1, rc[VE]))
                return red

            zpos = [i for i in range(PP)
                    if not any(r[0] <= i < r[1] for r in runs)]

            # ---- phase 4: layer-1 aggregate + h --------------------------
            gather_layer(y_full, pre_ve_wait=proj_ve)
            def h_ops(i0, i1, use_mean):
                def f(v):
                    a = htmp2[:, i0:i1, :]
                    b = htmp[:, i0:i1, :]
                    if use_mean:
                        v.tensor_tensor(out=a, in0=agg[:, i0:i1, :],
                                        in1=inv_sb[:, i0:i1, :],
                                        op=mybir.AluOpType.mult)
                        v.drain()
                        v.tensor_tensor(out=b, in0=a,
                                        in1=y1r[:, i0:i1, :],
                                        op=mybir.AluOpType.add)
                    else:
                        v.tensor_tensor(out=b, in0=agg[:, i0:i1, :],
                                        in1=y1r[:, i0:i1, :],
                                        op=mybir.AluOpType.add)
                    v.drain()
                    if with_b1:
                        v.tensor_tensor(out=a, in0=b,
                                        in1=b1_sb[:, i0:i1, :],
                                        op=mybir.AluOpType.add)
                        v.drain()
                        b = a
                    return v.tensor_scalar_max(hbuf[:, i0:i1, :], b, 0.0)
                return f
            for (i0, i1, _w) in runs:
                n_ops = 3 + (1 if with_b1 else 0)
                P.add("ve", h_ops(i0, i1, True), incs=[(VE, n_ops)])
            for i in zpos:
                n_ops = 2 + (1 if with_b1 else 0)
                P.add("ve", h_ops(i, i + 1, False), incs=[(VE, n_ops)])
            l1_ve = P.n(VE)

            # ---- phase 5: h writeback + AllGather + zero-fix -------------
            wh = P.add("sync",
                       gdma(h_slice[:, :].rearrange("(p i) f -> p i f", p=128),
                            hbuf[:, :, :]),
                       waits=[(VE, l1_ve)], incs=[(DS, 16)])
            wc2 = P.add("gp", coll(h_slice, h_full), waits=[(DS, wh[DS])],
                        incs=[(CC, 1)])
            P.add("gp", gdma(h_full[zero_g:zero_g + 1, :], z_sb[:, :]),
                  waits=[(CC, wc2[CC])], incs=[(DG, 16)])

            if dbg and _it == K - 1:
                P.add("sync", gdma(da1[:, :].rearrange("(p i) f -> p i f", p=128),
                                   agg[:, :, :]),
                      waits=[(VE, l1_ve)], incs=[(DS, 16)])
                # agg memset must additionally wait that dump
                P.add("ve", None, waits=[(DS, P.n(DS))])
            mz = P.add("ve", lambda v: (v.memset(agg[:, :, :], 0.0),
                                        v.drain())[0],
                       waits=[], incs=[(VE, 1)])
            pre2_ve = mz[VE]

            # ---- phase 6: layer-2 aggregate + final linear ---------------
            l2_red = gather_layer(h_full, pre_ve_wait=l1_ve)
            ocount = 0
            pst_free = [0, 0, 0, 0]  # VE count freeing psum-transpose bank
            pso_free = [0, 0, 0, 0]  # VE count freeing psum-out bank
            chunk_free = [0, 0, 0, 0]  # PE count freeing aTc/hTc pair

            def emit_out_chunk(i, ocount, mean_ve):
                cb = ocount % 4
                ob = ocount % 4
                tb = 2 * (ocount % 2)
                # transpose h[:, i, :] and agg[:, i, :] -> pst banks
                th = P.add("pe", transpose_op(pst[tb][:, :], hbuf[:, i, :]),
                           waits=[(VE, pre2_ve), (VE, pst_free[tb])],
                           incs=[(PE, 1)])
                ta = P.add("pe", transpose_op(pst[tb + 1][:, :], agg[:, i, :]),
                           waits=[(VE, mean_ve), (VE, pst_free[tb + 1])],
                           incs=[(PE, 1)])
                def cp_h(cb=cb, tb=tb):
                    return lambda v: v.tensor_copy(out=hTc[cb][:, :],
                                                   in_=pst[tb][:, :])
                def cp_a(cb=cb, tb=tb):
                    return lambda v: v.tensor_copy(out=aTc[cb][:, :],
                                                   in_=pst[tb + 1][:, :])
                ch = P.add("ve", cp_h(), waits=[(PE, th[PE]),
                                                (PE, chunk_free[cb])],
                           incs=[(VE, 1)])
                ca = P.add("ve", cp_a(), waits=[(PE, ta[PE])], incs=[(VE, 1)])
                pst_free[tb] = ch[VE]
                pst_free[tb + 1] = ca[VE]
                def mm_o1(cb=cb, ob=ob):
                    return lambda t: t.matmul(
                        out=pso[ob][:, :], lhsT=aTc[cb][:, :],
                        rhs=w2l_sb[:, :], start=True, stop=False)
                def mm_o2(cb=cb, ob=ob):
                    return lambda t: t.matmul(
                        out=pso[ob][:, :], lhsT=hTc[cb][:, :],
                        rhs=w2r_sb[:, :], start=False, stop=True)
                P.add("pe", mm_o1(),
                      waits=[(VE, ca[VE]), (VE, pso_free[ob])],
                      incs=[(PE, 1)])
                mo = P.add("pe", mm_o2(), incs=[(PE, 1)])
                chunk_free[cb] = mo[PE]
                def out_add(i=i, ob=ob):
                    if with_b2:
                        return lambda v: v.tensor_tensor(
                            out=out_sb[:, i, :], in0=pso[ob][:, :],
                            in1=b2_sb[:, :], op=mybir.AluOpType.add)
                    return lambda v: v.tensor_copy(out=out_sb[:, i, :],
                                                   in_=pso[ob][:, :])
                oa = P.add("ve", out_add(), waits=[(PE, mo[PE])], incs=[(VE, 1)])
                pso_free[ob] = oa[VE]

            for (i0, i1, rc) in l2_red:
                def mean_op(i0=i0, i1=i1):
                    def f(v):
                        r = v.tensor_tensor(
                            out=agg[:, i0:i1, :], in0=agg[:, i0:i1, :],
                            in1=inv_sb[:, i0:i1, :], op=mybir.AluOpType.mult)
                        v.drain()
                        return r
                    return f
                mc = P.add("ve", mean_op(), incs=[(VE, 1)])
                for i in range(i0, i1):
                    emit_out_chunk(i, ocount, mc[VE])
                    ocount += 1
            for i in zpos:
                emit_out_chunk(i, ocount, pre2_ve)
                ocount += 1

            fin_ve = P.n(VE)
            P.add("sync",
                  gdma(out_ext[:, :].rearrange("(p i) f -> p i f", p=128),
                       out_sb[:, :, :]),
                  waits=[(VE, fin_ve)], incs=[(DS, 16)])
            if dbg and _it == K - 1:
                P.add("sync", gdma(dy[:, :].rearrange("(p i) f -> p i f", p=128),
                                   y1l[:, :, :]),
                      waits=[(VE, fin_ve)], incs=[(DS, 16)])
                P.add("sync", gdma(dh[:, :].rearrange("(p i) f -> p i f", p=128),
                                   hbuf[:, :, :]),
                      waits=[(VE, fin_ve)], incs=[(DS, 16)])
                P.add("sync", gdma(da2[:, :].rearrange("(p i) f -> p i f", p=128),
                                   agg[:, :, :]),
                      waits=[(VE, fin_ve)], incs=[(DS, 16)])
                P.add("sync", gdma(dg1[:, :].rearrange("(p t) f -> p t f", p=128),
                                   G[:, 0:S // 8, :]),
                      waits=[(VE, fin_ve)], incs=[(DS, 16)])

            prev_end = {DG: P.n(DG), DS: P.n(DS), PE: P.n(PE),
                        VE: P.n(VE), CC: P.n(CC)}

        # ---- emission ----------------------------------------------------
        finals = [(k, P.n(k)) for k in (DG, DS, CC, PE, VE) if P.n(k) > 0]

        def emit_stream(eng_obj, key, own_sems):
            waited = {}
            def do_wait(sem_key, cnt):
                if cnt <= 0 or sem_key in own_sems:
                    return
                if waited.get(sem_key, -1) >= cnt:
                    return
                eng_obj.wait_ge(sems[sem_key], cnt)
                waited[sem_key] = cnt
            for op in P.ops[key]:
                for sem_key, cnt in op.waits:
                    do_wait(sem_key, cnt)
                if op.emit is None:
                    continue
                inst = op.emit(eng_obj)
                for sem_key, n_inc in op.incs:
                    inst.then_inc(sems[sem_key], n_inc)
            for sem_key, cnt in finals:
                if sem_key not in own_sems and waited.get(sem_key, -1) < cnt:
                    eng_obj.wait_ge(sems[sem_key], cnt)

        @block.gpsimd
        def _(g):
            emit_stream(g, "gp", own_sems=())

        @block.sync
        def _(s):
            emit_stream(s, "sync", own_sems=())

        @block.tensor
        def _(t):
            emit_stream(t, "pe", own_sems=("pe",))

        @block.vector
        def _(v):
            emit_stream(v, "ve", own_sems=("ve",))

    return nc


# ----------------------------------------------------------------------------
# PJRT runner (persistent jitted executable + device buffers)
# ----------------------------------------------------------------------------
class _Runner:
    def __init__(self, nc, n_cores=C):
        import jax
        from jax.sharding import Mesh, PartitionSpec
        from jax.experimental.shard_map import shard_map
        from concourse import mybir
        from concourse.bass2jax import (_bass_exec_p, install_neuronx_cc_hook,
                                        partition_id_tensor)

        install_neuronx_cc_hook()
        self.jax = jax
        self.n_cores = n_cores
        partition_name = (nc.partition_id_tensor.name
                          if nc.partition_id_tensor else None)
        in_names, out_names, out_avals, zero_outs = [], [], [], []
        for alloc in nc.m.functions[0].allocations:
            if not isinstance(alloc, mybir.MemoryLocationSet):
                continue
            name = alloc.memorylocations[0].name
            if alloc.kind == "ExternalInput":
                if name != partition_name:
                    in_names.append(name)
            elif alloc.kind == "ExternalOutput":
                out_names.append(name)
                shape = tuple(alloc.tensor_shape)
                dtype = mybir.dt.np(alloc.dtype)
                out_avals.append(jax.core.ShapedArray(shape, dtype))
                zero_outs.append(np.zeros(shape, dtype))
        self.in_names, self.out_names = in_names, out_names
        n_params = len(in_names)
        all_names = list(in_names) + list(out_names)
        if partition_name is not None:
            all_names.append(partition_name)

        def _body(*args):
            operands = list(args)
            if partition_name is not None:
                operands.append(partition_id_tensor())
            outs = _bass_exec_p.bind(
                *operands, out_avals=tuple(out_avals),
                in_names=tuple(all_names), out_names=tuple(out_names),
                lowering_input_output_aliases=(),
                sim_require_finite=False, sim_require_nnan=False, nc=nc)
            return tuple(outs)

        devices = jax.devices()[:n_cores]
        self.mesh = Mesh(np.asarray(devices), ("core",))
        in_specs = (PartitionSpec("core"),) * (n_params + len(out_names))
        out_specs = (PartitionSpec("core"),) * len(out_names)
        self.fn = jax.jit(
            shard_map(_body, mesh=self.mesh, in_specs=in_specs,
                      out_specs=out_specs, check_rep=False),
            keep_unused=True)
        self.zero_outs = zero_outs
        self.dev_args = None

    def stage(self, in_maps):
        import jax
        from jax.sharding import NamedSharding, PartitionSpec
        sharding = NamedSharding(self.mesh, PartitionSpec("core"))
        args = []
        for name in self.in_names:
            cat = np.concatenate(
                [np.ascontiguousarray(m[name]) for m in in_maps], axis=0)
            args.append(jax.device_put(cat, sharding))
        for z in self.zero_outs:
            cat = np.concatenate([z] * self.n_cores, axis=0)
            args.append(jax.device_put(cat, sharding))
        jax.block_until_ready(args)
        self.dev_args = args

    def run(self):
        outs = self.fn(*self.dev_args)
        self.jax.block_until_ready(outs)
        return outs

    def results(self, outs, name="out"):
        full = np.asarray(outs[self.out_names.index(name)])
        return np.split(full, self.n_cores, axis=0)


# ----------------------------------------------------------------------------
# public entry point
# ----------------------------------------------------------------------------
def _make_inputs(prep, x, W1_l, W1_r, b1, W2_l, W2_r, b2):
    orig_of = prep["orig_of"]
    x = np.asarray(x, np.float32)
    w1cat = np.concatenate([np.asarray(W1_l, np.float32).T,
                            np.asarray(W1_r, np.float32).T],
                           axis=1).astype(ml_dtypes.bfloat16)
    w2l = np.ascontiguousarray(np.asarray(W2_l, np.float32).T)
    w2r = np.ascontiguousarray(np.asarray(W2_r, np.float32).T)
    ident = np.eye(128, dtype=np.float32)
    zrow = np.zeros((1, F), np.float32)
    b1 = np.asarray(b1, np.float32)
    b2 = np.asarray(b2, np.float32)
    with_b1 = bool(np.any(b1))
    with_b2 = bool(np.any(b2))

    in_maps = []
    for c in range(C):
        ids = orig_of[c]
        real = ids < N
        xc_rank = np.zeros((PC, E_DIM), np.float32)
        xc_rank[real] = x[ids[real]]
        # rank r -> (p = r%128, i = r//128); stage x^T: xt[f, i, p]
        xt = np.ascontiguousarray(
            xc_rank.reshape(PP, 128, E_DIM).transpose(2, 0, 1)
        ).astype(ml_dtypes.bfloat16).reshape(128, PP * 128)
        m = {"xt": xt, "idx": prep["idx"][c], "inv": prep["inv8"][c],
             "w1cat": w1cat, "w2l": w2l, "w2r": w2r, "ident": ident,
             "zrow": zrow}
        if with_b1:
            m["b1rep"] = np.ascontiguousarray(
                np.broadcast_to(b1, (128, PP, F)), dtype=np.float32)
        if with_b2:
            m["b2rep"] = np.ascontiguousarray(
                np.broadcast_to(b2, (128, NCLS)), dtype=np.float32)
        in_maps.append(m)
    return in_maps, with_b1, with_b2


def _get_runner(prep, with_b1, with_b2, K=1, dbg=False):
    key = (prep["S"], tuple(prep["runs"]), with_b1, with_b2, K, dbg)
    if key not in _RUN_CACHE:
        nc = _build(prep["S"], prep["runs"], prep["t0"], prep["zero_g"],
                    K, with_b1, with_b2, dbg=dbg)
        _RUN_CACHE[key] = _Runner(nc)
    return _RUN_CACHE[key]


def _assemble(prep, per_core):
    orig_of = prep["orig_of"]
    out = np.zeros((N, NCLS), np.float32)
    for c in range(C):
        ids = orig_of[c]
        # device m order [p, i] -> rank order r = i*128 + p
        oc = per_core[c].reshape(128, PP, NCLS).transpose(1, 0, 2).reshape(
            PC, NCLS)
        real = ids < N
        out[ids[real]] = oc[real]
    return out


def kernel(x, edge_index, W1_l, W1_r, b1, W2_l, W2_r, b2):
    edge_index = np.asarray(edge_index)
    src = edge_index[0].astype(np.int64, copy=False)
    dst = edge_index[1].astype(np.int64, copy=False)
    prep = _preprocess(src, dst)
    in_maps, with_b1, with_b2 = _make_inputs(prep, x, W1_l, W1_r, b1,
                                             W2_l, W2_r, b2)
    runner = _get_runner(prep, with_b1, with_b2, K=1)
    runner.stage(in_maps)
    outs = runner.run()
    return _assemble(prep, runner.results(outs))



# revision 37
# speedup vs baseline: 1.1918x; 1.1918x over previous
"""GraphSAGE 2-layer forward on 8 Trainium2 NeuronCores (Bass kernel).

Strategy
--------
Math reordering: mean-aggregation commutes with the linear layers, so we
project first (y1 = x @ W1_l.T, 8 cols) and aggregate 8-wide messages
instead of 128-wide ones.

Sharding (dst-parallel): node n -> core n % 8 (keeps edge counts balanced
to ~0.2%).  Within a core, nodes are degree-sorted and dealt round-robin
onto the 128 SBUF partitions, so every partition holds the same
per-position degree sequence (padded to the max over all 1024 streams;
~2% overhead).  Each partition's edge stream is then a concatenation of
fixed-width runs, and the per-node segment-sum becomes ~21 regular
strided tensor_reduce instructions -- no scatter anywhere.

Per-edge data movement uses the only HW-correct indirect-DMA form in
this toolchain: 128 indices per instruction (one per partition), one
32-byte row each.  Inter-layer tables (y1, h; 400KB/core slices) are
shared via AllGather.  The tiny linear layers run on the PE; x is staged
host-pretransposed in bf16 so the projection needs no on-device
transposes, and the final linear runs on a 4-deep PSUM rotation.

Per-core pipeline: x load -> PE project -> AllGather y1 -> S indirect
gathers + interleaved run-reduces -> h = relu(...) -> AllGather h ->
S gathers + run-reduces -> PE final linear -> out.
"""

import numpy as np
import ml_dtypes

N = 100000
E_DIM = 128
F = 8
NCLS = 40
C = 8                 # cores
PP = 98               # node positions per partition
PC = 128 * PP         # node slots per core (12544)
NPAD = C * PC         # 100352
GATHER_LAG = 32       # sem margin for out-of-order DMA completions

_RUN_CACHE = {}


# ----------------------------------------------------------------------------
# host preprocessing: node permutation, edge slots, run structure
# ----------------------------------------------------------------------------
def _preprocess(src, dst):
    cnt = np.bincount(dst, minlength=N).astype(np.int64)
    cntp = np.zeros(NPAD, np.int64)
    cntp[:N] = cnt

    # per core: degree-sorted rank r; device slot m = (r%128)*PP + r//128
    orig_of = np.empty((C, PC), np.int64)     # [core, rank] -> original id
    g_of = np.empty(NPAD, np.int64)           # original id -> global m-index
    r = np.arange(PC)
    m_of_rank = (r % 128) * PP + r // 128
    for c in range(C):
        ids = np.arange(c, NPAD, C)
        order = np.argsort(cntp[ids], kind="stable")
        ids_sorted = ids[order]
        orig_of[c] = ids_sorted
        g_of[ids_sorted] = c * PC + m_of_rank

    # per-position padded width, unified over all cores/partitions
    deg_sorted = cntp[orig_of.reshape(-1)].reshape(C, PP, 128)  # [c, i, p]
    w = deg_sorted.max(axis=(0, 2)).astype(np.int64)            # [PP]
    S = int(w.sum())
    t0 = np.zeros(PP + 1, np.int64)
    np.cumsum(w, out=t0[1:])

    runs = []
    i = 0
    while i < PP:
        j = i
        while j < PP and w[j] == w[i]:
            j += 1
        if w[i] > 0:
            runs.append((int(i), int(j), int(w[i])))
        i = j

    # per-edge slot assignment
    g_dst = g_of[dst]
    order = np.argsort(g_dst)
    sg = g_dst[order]
    gsrc = g_of[src[order]].astype(np.int32)
    newgrp = np.empty(sg.shape[0], bool)
    newgrp[0] = True
    np.not_equal(sg[1:], sg[:-1], out=newgrp[1:])
    first_pos = np.flatnonzero(newgrp)
    grp = np.cumsum(newgrp) - 1
    k = np.arange(sg.shape[0]) - first_pos[grp]

    c_e = sg // PC
    m_e = sg % PC
    p_e = m_e // PP
    i_e = m_e % PP
    t_e = t0[i_e] + k

    # zero row: core 0's first pad node, in m-encoding
    pad_rank0 = int(np.flatnonzero(orig_of[0] >= N)[0])
    zero_g = int(m_of_rank[pad_rank0])        # core 0 base is 0

    idx = np.full((C, 128, S), zero_g, np.int32)
    idx[c_e, p_e, t_e] = gsrc

    inv = (1.0 / np.maximum(cntp, 1.0)).astype(np.float32)
    inv_cm = inv[orig_of]                                   # [C, rank]
    inv8 = np.ascontiguousarray(
        inv_cm.reshape(C, PP, 128).transpose(0, 2, 1))[..., None]  # [C,p,i,1]
    inv8 = np.ascontiguousarray(np.broadcast_to(inv8, (C, 128, PP, F)),
                                dtype=np.float32)

    return dict(S=S, runs=runs, t0=t0, idx=idx, inv8=inv8,
                orig_of=orig_of, zero_g=zero_g)


# ----------------------------------------------------------------------------
# program builder (raw Bass, explicit semaphore plan)
# ----------------------------------------------------------------------------
class _Op:
    __slots__ = ("emit", "waits", "incs")

    def __init__(self, emit, waits, incs):
        self.emit, self.waits, self.incs = emit, waits, incs


class _Plan:
    def __init__(self):
        self.ops = {k: [] for k in ("gp", "sync", "pe", "ve")}
        self.ct = {}

    def n(self, sem):
        return self.ct.get(sem, 0)

    def add(self, eng, emit, waits=(), incs=()):
        for sem, cnt in incs:
            self.ct[sem] = self.ct.get(sem, 0) + cnt
        self.ops[eng].append(_Op(emit, list(waits), list(incs)))
        return {sem: self.ct[sem] for sem, _ in incs}


def _build(S, runs, t0, zero_g, K, with_b1, with_b2, dbg=False):
    from concourse import bass, mybir
    from concourse.bass import IndirectOffsetOnAxis

    nc = bass.Bass(target_bir_lowering=False, debug=False)
    f32, i32 = mybir.dt.float32, mybir.dt.int32
    bf16 = mybir.dt.bfloat16

    x_in = nc.declare_dram_parameter("xt", [128, PP * 128], bf16, isOutput=False)
    idx_in = nc.declare_dram_parameter("idx", [128, S], i32, isOutput=False)
    inv_in = nc.declare_dram_parameter("inv", [128, PP, F], f32, isOutput=False)
    w1_in = nc.declare_dram_parameter("w1cat", [E_DIM, 16], bf16, isOutput=False)
    w2l_in = nc.declare_dram_parameter("w2l", [8, NCLS], f32, isOutput=False)
    w2r_in = nc.declare_dram_parameter("w2r", [8, NCLS], f32, isOutput=False)
    id_in = nc.declare_dram_parameter("ident", [128, 128], f32, isOutput=False)
    z_in = nc.declare_dram_parameter("zrow", [1, F], f32, isOutput=False)
    b1_in = b2_in = None
    if with_b1:
        b1_in = nc.declare_dram_parameter("b1rep", [128, PP, F], f32, isOutput=False)
    if with_b2:
        b2_in = nc.declare_dram_parameter("b2rep", [128, NCLS], f32, isOutput=False)
    out_ext = nc.declare_dram_parameter("out", [PC, NCLS], f32, isOutput=True)
    if dbg:
        dy = nc.declare_dram_parameter("dy", [PC, F], f32, isOutput=True)
        dh = nc.declare_dram_parameter("dh", [PC, F], f32, isOutput=True)
        da1 = nc.declare_dram_parameter("da1", [PC, F], f32, isOutput=True)
        da2 = nc.declare_dram_parameter("da2", [PC, F], f32, isOutput=True)
        dg1 = nc.declare_dram_parameter("dg1", [128 * (S // 8), F], f32, isOutput=True)

    y_slice = nc.dram_tensor("y_slice", [PC, F], f32)
    h_slice = nc.dram_tensor("h_slice", [PC, F], f32)
    y_full = nc.dram_tensor("y_full", [NPAD, F], f32, addr_space="Shared")
    h_full = nc.dram_tensor("h_full", [NPAD, F], f32, addr_space="Shared")

    P = _Plan()
    DG, DS, CC, PE, VE = "dg", "ds", "cc", "pe", "ve"

    from contextlib import ExitStack
    with ExitStack() as ctx:
        ec = ctx.enter_context
        block = ec(nc.Block())
        dg_s = ec(nc.semaphore("dg"))
        ds_s = ec(nc.semaphore("ds"))
        cc_s = ec(nc.semaphore("cc"))
        pe_s = ec(nc.semaphore("pe"))
        ve_s = ec(nc.semaphore("ve"))
        x_sb = ec(nc.sbuf_tensor("x_sb", [128, PP, 128], bf16))
        G = ec(nc.sbuf_tensor("G", [128, S, F], f32))
        idx_sb = ec(nc.sbuf_tensor("idx_sb", [128, S], i32))
        inv_sb = ec(nc.sbuf_tensor("inv_sb", [128, PP, F], f32))
        y1l = ec(nc.sbuf_tensor("y1l", [128, PP, F], f32))
        y1r = ec(nc.sbuf_tensor("y1r", [128, PP, F], f32))
        agg = ec(nc.sbuf_tensor("agg", [128, PP, F], f32))
        hbuf = ec(nc.sbuf_tensor("hbuf", [128, PP, F], f32))
        htmp = ec(nc.sbuf_tensor("htmp", [128, PP, F], f32))
        htmp2 = ec(nc.sbuf_tensor("htmp2", [128, PP, F], f32))
        out_sb = ec(nc.sbuf_tensor("out_sb", [128, PP, NCLS], f32))
        aTcs = [ec(nc.sbuf_tensor(f"aTc{m}", [8, 128], f32)) for m in range(4)]
        hTcs = [ec(nc.sbuf_tensor(f"hTc{m}", [8, 128], f32)) for m in range(4)]
        w1_sb = ec(nc.sbuf_tensor("w1_sb", [128, 16], bf16))
        w2l_sb = ec(nc.sbuf_tensor("w2l_sb", [8, NCLS], f32))
        w2r_sb = ec(nc.sbuf_tensor("w2r_sb", [8, NCLS], f32))
        id_sb = ec(nc.sbuf_tensor("id_sb", [128, 128], f32))
        z_sb = ec(nc.sbuf_tensor("z_sb", [1, F], f32))
        b1_sb = ec(nc.sbuf_tensor("b1_sb",
                                  [128, PP, F] if with_b1 else [1, F], f32))
        b2_sb = ec(nc.sbuf_tensor("b2_sb",
                                  [128, NCLS] if with_b2 else [1, F], f32))
        psts = [ec(nc.psum_tensor(f"pst{m}", [8, 128], f32)) for m in range(4)]
        psos = [ec(nc.psum_tensor(f"pso{m}", [128, NCLS], f32))
                for m in range(4)]
        sems = {DG: dg_s, DS: ds_s, CC: cc_s, PE: pe_s, VE: ve_s}
        aTc = aTcs
        hTc = hTcs
        psy = [psos[0], psos[1]]   # column-aliased: phases are disjoint
        pst = psts
        pso = psos

        def gdma(out, in_):
            return lambda g: g.dma_start(out=out, in_=in_)

        # ---- static loads -------------------------------------------------
        static_tiles = [
            (idx_sb[:, :], idx_in[:, :]),
            (inv_sb[:, :, :], inv_in[:, :, :]),
            (w1_sb[:, :], w1_in[:, :]),
            (w2l_sb[:, :], w2l_in[:, :]),
            (w2r_sb[:, :], w2r_in[:, :]),
            (id_sb[:, :], id_in[:, :]),
            (z_sb[:, :], z_in[:, :]),
        ]
        if with_b1:
            static_tiles.append((b1_sb[:, :, :], b1_in[:, :, :]))
        if with_b2:
            static_tiles.append((b2_sb[:, :], b2_in[:, :]))
        for out_t, in_t in static_tiles:
            P.add("gp", gdma(out_t, in_t), incs=[(DG, 16)])
        P.add("sync", gdma(x_sb[:, :, :],
                           x_in[:, :].rearrange("f (i n) -> f i n", i=PP)),
              incs=[(DS, 16)])
        static_dg = P.n(DG)
        static_ds = P.n(DS)

        def transpose_op(dst_ps, src_ap):
            return lambda t: t.matmul(out=dst_ps, lhsT=src_ap, rhs=id_sb[:, :],
                                      is_transpose=True, start=True, stop=True)

        def coll(inp, outp):
            return lambda g: g.collective_compute(
                "AllGather", mybir.AluOpType.bypass,
                replica_groups=[list(range(C))],
                ins=[inp.ap().opt()], outs=[outp.ap().opt()])

        prev_end = {DG: static_dg, DS: static_ds, PE: 0, VE: 0, CC: 0}

        for _it in range(K):
            # ---- body-start barrier --------------------------------------
            bar = [(s, prev_end[s]) for s in (DG, DS, PE, VE, CC)]
            for eng in ("gp", "sync", "pe", "ve"):
                P.add(eng, None, waits=bar)

            # ---- phase 2: project x (PE/VE pipeline) ---------------------
            pe_base = P.n(PE)
            ve_base = P.n(VE)
            for i in range(PP):
                waits_t = [(DS, static_ds)]
                if i == 0:
                    waits_t.append((DG, static_dg))
                if i >= 2:
                    waits_t.append((VE, ve_base + 2 * (i - 2) + 2))

                def mm_y(i=i):
                    return lambda t: t.matmul(out=psy[i % 2][:, 0:16],
                                              lhsT=x_sb[:, i, :],
                                              rhs=w1_sb[:, :],
                                              start=True, stop=True)
                P.add("pe", mm_y(), waits=waits_t, incs=[(PE, 1)])

                def cp_yl(i=i):
                    return lambda v: v.tensor_copy(out=y1l[:, i, :],
                                                   in_=psy[i % 2][:, 0:8])
                def cp_yr(i=i):
                    return lambda v: v.tensor_copy(out=y1r[:, i, :],
                                                   in_=psy[i % 2][:, 8:16])
                P.add("ve", cp_yl(), waits=[(PE, pe_base + i + 1)],
                      incs=[(VE, 1)])
                P.add("ve", cp_yr(), incs=[(VE, 1)])

            P.add("ve", lambda v: (v.memset(agg[:, :, :], 0.0), v.drain())[0],
                  incs=[(VE, 1)])
            proj_ve = P.n(VE)

            # ---- phase 3: y writeback + AllGather + zero-fix -------------
            wy = P.add("sync",
                       gdma(y_slice[:, :].rearrange("(p i) f -> p i f", p=128),
                            y1l[:, :, :]),
                       waits=[(VE, proj_ve)], incs=[(DS, 16)])
            wc = P.add("gp", coll(y_slice, y_full), waits=[(DS, wy[DS])],
                       incs=[(CC, 1)])

            # ---- gather+reduce helper ------------------------------------
            # post_cb(i0, i1, wd) is emitted right after each run's reduce
            # so the VE stream processes post work progressively while the
            # gather stream continues (VE executes in order).
            def gather_layer(table, pre_ve_wait, cc_wait, post_cb=None):
                base_dg = P.n(DG)
                for t in range(S):
                    def gop(t=t, table=table):
                        return lambda g: g.indirect_dma_start(
                            out=G[:, t, :], out_offset=None, in_=table[:, :],
                            in_offset=IndirectOffsetOnAxis(
                                ap=idx_sb[:, t:t + 1], axis=0))
                    wts = ([(DG, base_dg), (VE, pre_ve_wait), (CC, cc_wait)]
                           if t == 0 else [])
                    P.add("gp", gop(), waits=wts, incs=[(DG, 16)])
                for (i0, i1, wd) in runs:
                    thresh = base_dg + 16 * min(S, int(t0[i1]) + GATHER_LAG)
                    def rop(i0=i0, i1=i1, wd=wd):
                        lo, hi = int(t0[i0]), int(t0[i1])
                        def f(v):
                            src = G[:, lo:hi, :].rearrange(
                                "p (i w) f -> p i f w", w=wd)
                            return v.tensor_reduce(
                                out=agg[:, i0:i1, :], in_=src,
                                axis=mybir.AxisListType.X,
                                op=mybir.AluOpType.add)
                        return f
                    P.add("ve", rop(), waits=[(DG, thresh)], incs=[(VE, 1)])
                    if post_cb is not None:
                        post_cb(i0, i1, wd)

            zpos = [i for i in range(PP)
                    if not any(r[0] <= i < r[1] for r in runs)]

            # ---- phase 4: layer-1 aggregate + h --------------------------
            def h_ops(i0, i1, use_mean):
                def f(v):
                    a = htmp2[:, i0:i1, :]
                    b = htmp[:, i0:i1, :]
                    if use_mean:
                        v.drain()
                        v.tensor_tensor(out=a, in0=agg[:, i0:i1, :],
                                        in1=inv_sb[:, i0:i1, :],
                                        op=mybir.AluOpType.mult)
                        v.drain()
                        v.tensor_tensor(out=b, in0=a,
                                        in1=y1r[:, i0:i1, :],
                                        op=mybir.AluOpType.add)
                    else:
                        v.tensor_tensor(out=b, in0=agg[:, i0:i1, :],
                                        in1=y1r[:, i0:i1, :],
                                        op=mybir.AluOpType.add)
                    v.drain()
                    if with_b1:
                        v.tensor_tensor(out=a, in0=b,
                                        in1=b1_sb[:, i0:i1, :],
                                        op=mybir.AluOpType.add)
                        v.drain()
                        b = a
                    return v.tensor_scalar_max(hbuf[:, i0:i1, :], b, 0.0)
                return f
            for i in zpos:
                n_ops = 2 + (1 if with_b1 else 0)
                P.add("ve", h_ops(i, i + 1, False), incs=[(VE, n_ops)])

            def l1_post(i0, i1, wd):
                n_ops = 3 + (1 if with_b1 else 0)
                P.add("ve", h_ops(i0, i1, True), incs=[(VE, n_ops)])
            gather_layer(y_full, pre_ve_wait=proj_ve, cc_wait=wc[CC],
                         post_cb=l1_post)
            l1_ve = P.n(VE)

            # ---- phase 5: h writeback + AllGather + zero-fix -------------
            wh = P.add("sync",
                       gdma(h_slice[:, :].rearrange("(p i) f -> p i f", p=128),
                            hbuf[:, :, :]),
                       waits=[(VE, l1_ve)], incs=[(DS, 16)])
            wc2 = P.add("gp", coll(h_slice, h_full), waits=[(DS, wh[DS])],
                        incs=[(CC, 1)])
            if with_b1:
                # pad rows of h are relu(b1) != 0 only when b1 is nonzero
                P.add("gp", gdma(h_full[zero_g:zero_g + 1, :], z_sb[:, :]),
                      waits=[(CC, wc2[CC])], incs=[(DG, 16)])

            if dbg and _it == K - 1:
                P.add("sync", gdma(da1[:, :].rearrange("(p i) f -> p i f", p=128),
                                   agg[:, :, :]),
                      waits=[(VE, l1_ve)], incs=[(DS, 16)])
                # agg memset must additionally wait that dump
                P.add("ve", None, waits=[(DS, P.n(DS))])
            mz = P.add("ve", lambda v: (v.memset(agg[:, :, :], 0.0),
                                        v.drain())[0],
                       waits=[], incs=[(VE, 1)])
            pre2_ve = mz[VE]

            # ---- phase 6: layer-2 aggregate + final linear ---------------
            oc = [0]
            pst_free = [0, 0, 0, 0]  # VE count freeing psum-transpose bank
            pso_free = [0, 0, 0, 0]  # VE count freeing psum-out bank
            chunk_free = [0, 0, 0, 0]  # PE count freeing aTc/hTc pair

            def emit_out_chunk(i, ocount, mean_ve):
                cb = ocount % 4
                ob = ocount % 4
                tb = 2 * (ocount % 2)
                # transpose h[:, i, :] and agg[:, i, :] -> pst banks
                th = P.add("pe", transpose_op(pst[tb][:, :], hbuf[:, i, :]),
                           waits=[(VE, pre2_ve), (VE, pst_free[tb])],
                           incs=[(PE, 1)])
                ta = P.add("pe", transpose_op(pst[tb + 1][:, :], agg[:, i, :]),
                           waits=[(VE, mean_ve), (VE, pst_free[tb + 1])],
                           incs=[(PE, 1)])
                def cp_h(cb=cb, tb=tb):
                    return lambda v: v.tensor_copy(out=hTc[cb][:, :],
                                                   in_=pst[tb][:, :])
                def cp_a(cb=cb, tb=tb):
                    return lambda v: v.tensor_copy(out=aTc[cb][:, :],
                                                   in_=pst[tb + 1][:, :])
                ch = P.add("ve", cp_h(), waits=[(PE, th[PE]),
                                                (PE, chunk_free[cb])],
                           incs=[(VE, 1)])
                ca = P.add("ve", cp_a(), waits=[(PE, ta[PE])], incs=[(VE, 1)])
                pst_free[tb] = ch[VE]
                pst_free[tb + 1] = ca[VE]
                def mm_o1(cb=cb, ob=ob):
                    return lambda t: t.matmul(
                        out=pso[ob][:, :], lhsT=aTc[cb][:, :],
                        rhs=w2l_sb[:, :], start=True, stop=False)
                def mm_o2(cb=cb, ob=ob):
                    return lambda t: t.matmul(
                        out=pso[ob][:, :], lhsT=hTc[cb][:, :],
                        rhs=w2r_sb[:, :], start=False, stop=True)
                P.add("pe", mm_o1(),
                      waits=[(VE, ca[VE]), (VE, pso_free[ob])],
                      incs=[(PE, 1)])
                mo = P.add("pe", mm_o2(), incs=[(PE, 1)])
                chunk_free[cb] = mo[PE]
                def out_add(i=i, ob=ob):
                    if with_b2:
                        return lambda v: v.tensor_tensor(
                            out=out_sb[:, i, :], in0=pso[ob][:, :],
                            in1=b2_sb[:, :], op=mybir.AluOpType.add)
                    return lambda v: v.tensor_copy(out=out_sb[:, i, :],
                                                   in_=pso[ob][:, :])
                oa = P.add("ve", out_add(), waits=[(PE, mo[PE])], incs=[(VE, 1)])
                pso_free[ob] = oa[VE]

            for i in zpos:
                emit_out_chunk(i, oc[0], pre2_ve)
                oc[0] += 1

            def l2_post(i0, i1, wd):
                def mean_op(i0=i0, i1=i1):
                    def f(v):
                        v.drain()
                        r = v.tensor_tensor(
                            out=agg[:, i0:i1, :], in0=agg[:, i0:i1, :],
                            in1=inv_sb[:, i0:i1, :], op=mybir.AluOpType.mult)
                        v.drain()
                        return r
                    return f
                mc = P.add("ve", mean_op(), incs=[(VE, 1)])
                for i in range(i0, i1):
                    emit_out_chunk(i, oc[0], mc[VE])
                    oc[0] += 1
            gather_layer(h_full, pre_ve_wait=l1_ve, cc_wait=wc2[CC],
                         post_cb=l2_post)

            fin_ve = P.n(VE)
            P.add("sync",
                  gdma(out_ext[:, :].rearrange("(p i) f -> p i f", p=128),
                       out_sb[:, :, :]),
                  waits=[(VE, fin_ve)], incs=[(DS, 16)])
            if dbg and _it == K - 1:
                P.add("sync", gdma(dy[:, :].rearrange("(p i) f -> p i f", p=128),
                                   y1l[:, :, :]),
                      waits=[(VE, fin_ve)], incs=[(DS, 16)])
                P.add("sync", gdma(dh[:, :].rearrange("(p i) f -> p i f", p=128),
                                   hbuf[:, :, :]),
                      waits=[(VE, fin_ve)], incs=[(DS, 16)])
                P.add("sync", gdma(da2[:, :].rearrange("(p i) f -> p i f", p=128),
                                   agg[:, :, :]),
                      waits=[(VE, fin_ve)], incs=[(DS, 16)])
                P.add("sync", gdma(dg1[:, :].rearrange("(p t) f -> p t f", p=128),
                                   G[:, 0:S // 8, :]),
                      waits=[(VE, fin_ve)], incs=[(DS, 16)])

            prev_end = {DG: P.n(DG), DS: P.n(DS), PE: P.n(PE),
                        VE: P.n(VE), CC: P.n(CC)}

        # ---- emission ----------------------------------------------------
        finals = [(k, P.n(k)) for k in (DG, DS, CC, PE, VE) if P.n(k) > 0]

        def emit_stream(eng_obj, key, own_sems):
            waited = {}
            def do_wait(sem_key, cnt):
                if cnt <= 0 or sem_key in own_sems:
                    return
                if waited.get(sem_key, -1) >= cnt:
                    return
                eng_obj.wait_ge(sems[sem_key], cnt)
                waited[sem_key] = cnt
            for op in P.ops[key]:
                for sem_key, cnt in op.waits:
                    do_wait(sem_key, cnt)
                if op.emit is None:
                    continue
                inst = op.emit(eng_obj)
                for sem_key, n_inc in op.incs:
                    inst.then_inc(sems[sem_key], n_inc)
            for sem_key, cnt in finals:
                if sem_key not in own_sems and waited.get(sem_key, -1) < cnt:
                    eng_obj.wait_ge(sems[sem_key], cnt)

        @block.gpsimd
        def _(g):
            emit_stream(g, "gp", own_sems=())

        @block.sync
        def _(s):
            emit_stream(s, "sync", own_sems=())

        @block.tensor
        def _(t):
            emit_stream(t, "pe", own_sems=("pe",))

        @block.vector
        def _(v):
            emit_stream(v, "ve", own_sems=("ve",))

    return nc


# ----------------------------------------------------------------------------
# PJRT runner (persistent jitted executable + device buffers)
# ----------------------------------------------------------------------------
class _Runner:
    def __init__(self, nc, n_cores=C):
        import jax
        from jax.sharding import Mesh, PartitionSpec
        from jax.experimental.shard_map import shard_map
        from concourse import mybir
        from concourse.bass2jax import (_bass_exec_p, install_neuronx_cc_hook,
                                        partition_id_tensor)

        install_neuronx_cc_hook()
        self.jax = jax
        self.n_cores = n_cores
        partition_name = (nc.partition_id_tensor.name
                          if nc.partition_id_tensor else None)
        in_names, out_names, out_avals, zero_outs = [], [], [], []
        for alloc in nc.m.functions[0].allocations:
            if not isinstance(alloc, mybir.MemoryLocationSet):
                continue
            name = alloc.memorylocations[0].name
            if alloc.kind == "ExternalInput":
                if name != partition_name:
                    in_names.append(name)
            elif alloc.kind == "ExternalOutput":
                out_names.append(name)
                shape = tuple(alloc.tensor_shape)
                dtype = mybir.dt.np(alloc.dtype)
                out_avals.append(jax.core.ShapedArray(shape, dtype))
                zero_outs.append(np.zeros(shape, dtype))
        self.in_names, self.out_names = in_names, out_names
        n_params = len(in_names)
        all_names = list(in_names) + list(out_names)
        if partition_name is not None:
            all_names.append(partition_name)

        def _body(*args):
            operands = list(args)
            if partition_name is not None:
                operands.append(partition_id_tensor())
            outs = _bass_exec_p.bind(
                *operands, out_avals=tuple(out_avals),
                in_names=tuple(all_names), out_names=tuple(out_names),
                lowering_input_output_aliases=(),
                sim_require_finite=False, sim_require_nnan=False, nc=nc)
            return tuple(outs)

        devices = jax.devices()[:n_cores]
        self.mesh = Mesh(np.asarray(devices), ("core",))
        in_specs = (PartitionSpec("core"),) * (n_params + len(out_names))
        out_specs = (PartitionSpec("core"),) * len(out_names)
        self.fn = jax.jit(
            shard_map(_body, mesh=self.mesh, in_specs=in_specs,
                      out_specs=out_specs, check_rep=False),
            keep_unused=True)
        self.zero_outs = zero_outs
        self.dev_args = None

    def stage(self, in_maps):
        import jax
        from jax.sharding import NamedSharding, PartitionSpec
        sharding = NamedSharding(self.mesh, PartitionSpec("core"))
        args = []
        for name in self.in_names:
            cat = np.concatenate(
                [np.ascontiguousarray(m[name]) for m in in_maps], axis=0)
            args.append(jax.device_put(cat, sharding))
        for z in self.zero_outs:
            cat = np.concatenate([z] * self.n_cores, axis=0)
            args.append(jax.device_put(cat, sharding))
        jax.block_until_ready(args)
        self.dev_args = args

    def run(self):
        outs = self.fn(*self.dev_args)
        self.jax.block_until_ready(outs)
        return outs

    def results(self, outs, name="out"):
        full = np.asarray(outs[self.out_names.index(name)])
        return np.split(full, self.n_cores, axis=0)


# ----------------------------------------------------------------------------
# public entry point
# ----------------------------------------------------------------------------
def _make_inputs(prep, x, W1_l, W1_r, b1, W2_l, W2_r, b2):
    orig_of = prep["orig_of"]
    x = np.asarray(x, np.float32)
    w1cat = np.concatenate([np.asarray(W1_l, np.float32).T,
                            np.asarray(W1_r, np.float32).T],
                           axis=1).astype(ml_dtypes.bfloat16)
    w2l = np.ascontiguousarray(np.asarray(W2_l, np.float32).T)
    w2r = np.ascontiguousarray(np.asarray(W2_r, np.float32).T)
    ident = np.eye(128, dtype=np.float32)
    zrow = np.zeros((1, F), np.float32)
    b1 = np.asarray(b1, np.float32)
    b2 = np.asarray(b2, np.float32)
    with_b1 = bool(np.any(b1))
    with_b2 = bool(np.any(b2))

    in_maps = []
    for c in range(C):
        ids = orig_of[c]
        real = ids < N
        xc_rank = np.zeros((PC, E_DIM), np.float32)
        xc_rank[real] = x[ids[real]]
        # rank r -> (p = r%128, i = r//128); stage x^T: xt[f, i, p]
        xt = np.ascontiguousarray(
            xc_rank.reshape(PP, 128, E_DIM).transpose(2, 0, 1)
        ).astype(ml_dtypes.bfloat16).reshape(128, PP * 128)
        m = {"xt": xt, "idx": prep["idx"][c], "inv": prep["inv8"][c],
             "w1cat": w1cat, "w2l": w2l, "w2r": w2r, "ident": ident,
             "zrow": zrow}
        if with_b1:
            m["b1rep"] = np.ascontiguousarray(
                np.broadcast_to(b1, (128, PP, F)), dtype=np.float32)
        if with_b2:
            m["b2rep"] = np.ascontiguousarray(
                np.broadcast_to(b2, (128, NCLS)), dtype=np.float32)
        in_maps.append(m)
    return in_maps, with_b1, with_b2


def _get_runner(prep, with_b1, with_b2, K=1, dbg=False):
    key = (prep["S"], tuple(prep["runs"]), with_b1, with_b2, K, dbg)
    if key not in _RUN_CACHE:
        nc = _build(prep["S"], prep["runs"], prep["t0"], prep["zero_g"],
                    K, with_b1, with_b2, dbg=dbg)
        _RUN_CACHE[key] = _Runner(nc)
    return _RUN_CACHE[key]


def _assemble(prep, per_core):
    orig_of = prep["orig_of"]
    out = np.zeros((N, NCLS), np.float32)
    for c in range(C):
        ids = orig_of[c]
        # device m order [p, i] -> rank order r = i*128 + p
        oc = per_core[c].reshape(128, PP, NCLS).transpose(1, 0, 2).reshape(
            PC, NCLS)
        real = ids < N
        out[ids[real]] = oc[real]
    return out


def kernel(x, edge_index, W1_l, W1_r, b1, W2_l, W2_r, b2):
    edge_index = np.asarray(edge_index)
    src = edge_index[0].astype(np.int64, copy=False)
    dst = edge_index[1].astype(np.int64, copy=False)
    prep = _preprocess(src, dst)
    in_maps, with_b1, with_b2 = _make_inputs(prep, x, W1_l, W1_r, b1,
                                             W2_l, W2_r, b2)
    runner = _get_runner(prep, with_b1, with_b2, K=1)
    runner.stage(in_maps)
    outs = runner.run()
    return _assemble(prep, runner.results(outs))

